# revision 21
# baseline (speedup 1.0000x reference)
"""LIF spike scan kernel for Trainium2, SPMD over 8 NeuronCores.

Problem: x [B=64, T=8, C=128, H=32, W=32] f32.  Per (b,c,h,w) pixel, scan
over T:  v = tau*u + x_t ; s_t = (v > 1) ; u = v*(v <= 1).  Output spikes
[B, T, C, H, W] f32.

Design: all-int16 scaled domain; the scan itself is 3 packed DVE ops per
step, and the output is bit-packed by the (otherwise idle) PE+ACT engines
so HBM write traffic drops 8x.  The recurrence is scale-invariant, so the
host ships q = round(x * 2^12) int16 and the device scans integer membrane
state (threshold 4096 = 1.0).  Per step, with m = tau*u the pre-halved
carry:
    v = m + q_t               DVE tensor_tensor add   i16 x i16 -> 2x_1P
    g = (v <= 4096) * 0.5     DVE tensor_scalar dual  i16 -> fp16 {0,0.5} 4x
    m = v * g                 DVE tensor_tensor mult  i16 x fp16 -> 2x_1P
    pack += 2^(tl+1) * g      PE matmul diag(2^(tl+1)) @ g -> f32 PSUM
After 4 steps the PSUM byte-plane holds sum(2^tl * keep_tl) in [0,15]; ACT
copies it to uint8 and one small DMA ships it.  Host decodes
spike(t=h*4+tl) = 1 - bit tl of byte[h].  Groups run in pairs so two
[C,2048] f32 pack accumulators exactly fill the 8 PSUM banks.
tau=0.5 keeps v dyadic, compares vs 4096 are exact, i16 writeback
saturates and rounds-to-nearest-even (hw-verified): 2202 flipped spikes
of 9.3M vs the f32 reference (rel 1.54e-2 < 2e-2 gate) from input
quantization + halving ties.

Sharding: pure batch-parallel across 8 cores, no collectives.
"""

import numpy as np

B, T, C, HW = 64, 8, 128, 32 * 32
N_CORES = 8
B_LOC = B // N_CORES
SCALE = 2.0 ** -12
THI = 4096.0  # threshold in scaled domain
GB = 2        # batch rows per scan group (F = GB*HW = 2048 free dim)
NG = B_LOC // GB
TH = T // 2   # t-steps per half-chunk

_cache = {}


def _build_nc():
    from concourse import bacc, mybir, tile

    op = mybir.AluOpType
    nc = bacc.Bacc(
        "TRN2", target_bir_lowering=False, debug=False, num_devices=N_CORES
    )
    i16, f16, f32 = mybir.dt.int16, mybir.dt.float16, mybir.dt.float32
    u8 = mybir.dt.uint8
    F = GB * HW
    # q pre-shuffled on host to [g*2+h, c, (tl bl hw)]: contiguous 2D loads.
    x_ext = nc.dram_tensor(
        "x", [NG * 2, C, TH * F], i16, kind="ExternalInput"
    ).ap()
    # Pack weights: w[:, tl*C:(tl+1)*C] = 2^(tl+1) * I  (fp16, exact)
    w_ext = nc.dram_tensor(
        "w", [C, TH * C], f16, kind="ExternalInput"
    ).ap()
    # Output: one byte-plane per (group, half): bit tl = keep at t=h*4+tl.
    out_ext = nc.dram_tensor(
        "out", [NG * 2, C, F], u8, kind="ExternalOutput"
    ).ap()

    with tile.TileContext(nc) as tc:
        with tc.tile_pool(name="pool", bufs=2) as pool, tc.tile_pool(
            name="psum", bufs=2, space="PSUM"
        ) as ppool:
            wt = pool.tile([C, TH * C], f16, tag="w", bufs=1)
            # load pack weights on the Scalar queue so the first x chunk
            # owns the Sync queue from cycle 0
            nc.scalar.dma_start(out=wt, in_=w_ext)
            # Per-group membrane carry m = tau*u, persists across halves.
            mt = [
                pool.tile([C, F], i16, tag=f"m{g}", bufs=1, name=f"m{g}")
                for g in range(NG)
            ]
            xc = {}
            for h in range(2):
                for pair in range(NG // 2):
                    gs = (2 * pair, 2 * pair + 1)
                    for g in gs:
                        xc[g] = pool.tile(
                            [C, TH * F], i16, tag="x", bufs=6, name=f"x{h}_{g}"
                        )
                    # breadth-first quarter loads; the very first slice of
                    # the kernel is split 4x finer so the DVE can start on
                    # the first 128KB instead of waiting for 512KB
                    first = h == 0 and pair == 0
                    for tl in range(TH):
                        for g in gs:
                            lo = tl * F
                            if first and tl == 0 and g == gs[0]:
                                for j in range(0, F, 512):
                                    nc.sync.dma_start(
                                        out=xc[g][:, lo + j : lo + j + 512],
                                        in_=x_ext[
                                            g * 2 + h, :, lo + j : lo + j + 512
                                        ],
                                    )
                            else:
                                nc.sync.dma_start(
                                    out=xc[g][:, lo : lo + F],
                                    in_=x_ext[g * 2 + h, :, lo : lo + F],
                                )
                    pk = {
                        g: ppool.tile([C, F], f32, tag="pk", name=f"pk{h}_{g}")
                        for g in gs
                    }
                    for tl in range(TH):
                        t = h * TH + tl
                        vs = {g: xc[g][:, tl * F : (tl + 1) * F] for g in gs}
                        gt = {
                            g: pool.tile(
                                [C, F], f16, tag="g", bufs=8, name=f"g{t}_{g}"
                            )
                            for g in gs
                        }
                        if t > 0:
                            for g in gs:
                                # v = m + q_t (in place; i16 2x)
                                nc.vector.tensor_tensor(
                                    out=vs[g], in0=mt[g], in1=vs[g], op=op.add
                                )
                        for g in gs:
                            # keep-gate with tau folded in: {0,0.5} fp16 (4x)
                            if first and tl == 0 and g == gs[0]:
                                for j in range(0, F, 512):
                                    nc.vector.tensor_scalar(
                                        out=gt[g][:, j : j + 512],
                                        in0=vs[g][:, j : j + 512],
                                        scalar1=THI, scalar2=0.5,
                                        op0=op.is_le, op1=op.mult,
                                    )
                            else:
                                nc.vector.tensor_scalar(
                                    out=gt[g], in0=vs[g], scalar1=THI,
                                    scalar2=0.5, op0=op.is_le, op1=op.mult,
                                )
                        if t < T - 1:
                            for g in gs:
                                # m = v * g  (reset + tau; i16 x fp16 2x)
                                nc.vector.tensor_tensor(
                                    out=mt[g], in0=vs[g], in1=gt[g], op=op.mult
                                )
                        for g in gs:
                            # pack += 2^(tl+1) * g  (PE, f32 PSUM, exact;
                            # moving free dim capped at 512)
                            for j in range(0, F, 512):
                                nc.tensor.matmul(
                                    pk[g][:, j : j + 512],
                                    wt[:, tl * C : (tl + 1) * C],
                                    gt[g][:, j : j + 512],
                                    start=(tl == 0),
                                    stop=(tl == TH - 1),
                                )
                    last = h == 1 and pair == NG // 2 - 1
                    for gi, g in enumerate(gs):
                        pu = pool.tile(
                            [C, F], u8, tag="pu", bufs=4, name=f"p{h}{g}"
                        )
                        if last and gi == 1:
                            # run the final PSUM->u8 copy on the DVE so it
                            # overlaps the other group's ACT copy at the tail
                            nc.vector.tensor_copy(out=pu, in_=pk[g])
                        else:
                            nc.scalar.copy(out=pu, in_=pk[g])
                        nc.scalar.dma_start(out=out_ext[g * 2 + h], in_=pu)
    nc.compile()
    return nc


def _run(x: np.ndarray, trace: bool = False, tmpdir=None):
    from concourse.bass_utils import run_bass_kernel_spmd

    if "nc" not in _cache:
        _cache["nc"] = _build_nc()
    nc = _cache["nc"]
    x = np.asarray(x)
    q = np.clip(np.rint(x * np.float32(1.0 / SCALE)), -32768, 32767).astype(
        np.int16
    )
    # q[b=(g*GB+bl), t=(h*TH+tl), c, hw] -> [core, g, h, c, tl, bl, hw]
    q6 = q.reshape(N_CORES, NG, GB, 2, TH, C, HW)
    q_shuf = np.ascontiguousarray(q6.transpose(0, 1, 3, 5, 4, 2, 6)).reshape(
        N_CORES, NG * 2, C, TH * GB * HW
    )
    w = np.zeros((C, TH * C), dtype=np.float16)
    for tl in range(TH):
        w[np.arange(C), tl * C + np.arange(C)] = np.float16(2.0 ** (tl + 1))
    in_maps = [{"x": q_shuf[i], "w": w} for i in range(N_CORES)]
    res = run_bass_kernel_spmd(
        nc, in_maps, core_ids=list(range(N_CORES)), trace=trace, tmpdir=tmpdir
    )
    _cache["last_results"] = res
    outs = [res.results[i]["out"] for i in range(N_CORES)]
    # bytes [core, g*2+h, c, (bl hw)]; bit tl = keep at t = h*4+tl
    by = np.stack(outs, axis=0).reshape(N_CORES, NG, 2, 1, C, GB, HW)
    by = by.astype(np.uint8)
    tl_idx = np.arange(TH, dtype=np.uint8).reshape(1, 1, 1, TH, 1, 1, 1)
    keep = (by >> tl_idx) & np.uint8(1)           # [core, g, h, tl, c, bl, hw]
    spk = (1 - keep).astype(np.float32)
    out = spk.transpose(0, 1, 5, 2, 3, 4, 6).reshape(B, T, C, HW)
    return np.ascontiguousarray(out).reshape(B, T, C, 32, 32)


def kernel(x: np.ndarray) -> np.ndarray:
    return _run(x, trace=False)


# revision 22
# speedup vs baseline: 1.0083x; 1.0083x over previous
"""LIF spike scan kernel for Trainium2, SPMD over 8 NeuronCores.

Problem: x [B=64, T=8, C=128, H=32, W=32] f32.  Per (b,c,h,w) pixel, scan
over T:  v = tau*u + x_t ; s_t = (v > 1) ; u = v*(v <= 1).  Output spikes
[B, T, C, H, W] f32.

Design: all-int16 scaled domain; the scan itself is 3 packed DVE ops per
step, and the output is bit-packed by the (otherwise idle) PE+ACT engines
so HBM write traffic drops 8x.  The recurrence is scale-invariant, so the
host ships q = round(x * 2^12) int16 and the device scans integer membrane
state (threshold 4096 = 1.0).  Per step, with m = tau*u the pre-halved
carry:
    v = m + q_t               DVE tensor_tensor add   i16 x i16 -> 2x_1P
    g = (v <= 4096) * 0.5     DVE tensor_scalar dual  i16 -> fp16 {0,0.5} 4x
    m = v * g                 DVE tensor_tensor mult  i16 x fp16 -> 2x_1P
    pack += 2^(tl+1) * g      PE matmul diag(2^(tl+1)) @ g -> f32 PSUM
After 4 steps the PSUM byte-plane holds sum(2^tl * keep_tl) in [0,15]; ACT
copies it to uint8 and one small DMA ships it.  Host decodes
spike(t=h*4+tl) = 1 - bit tl of byte[h].  Groups run in pairs so two
[C,2048] f32 pack accumulators exactly fill the 8 PSUM banks.
tau=0.5 keeps v dyadic, compares vs 4096 are exact, i16 writeback
saturates and rounds-to-nearest-even (hw-verified): 2202 flipped spikes
of 9.3M vs the f32 reference (rel 1.54e-2 < 2e-2 gate) from input
quantization + halving ties.

Sharding: pure batch-parallel across 8 cores, no collectives.
"""

import numpy as np

B, T, C, HW = 64, 8, 128, 32 * 32
N_CORES = 8
B_LOC = B // N_CORES
SCALE = 2.0 ** -12
THI = 4096.0  # threshold in scaled domain
GB = 2        # batch rows per scan group (F = GB*HW = 2048 free dim)
NG = B_LOC // GB
TH = T // 2   # t-steps per half-chunk

_cache = {}


def _build_nc():
    from concourse import bacc, mybir, tile

    op = mybir.AluOpType
    nc = bacc.Bacc(
        "TRN2", target_bir_lowering=False, debug=False, num_devices=N_CORES
    )
    i16, f16, f32 = mybir.dt.int16, mybir.dt.float16, mybir.dt.float32
    u8 = mybir.dt.uint8
    F = GB * HW
    # q pre-shuffled on host to [g*2+h, c, (tl bl hw)]: contiguous 2D loads.
    x_ext = nc.dram_tensor(
        "x", [NG * 2, C, TH * F], i16, kind="ExternalInput"
    ).ap()
    # Pack weights: w[:, tl*C:(tl+1)*C] = 2^(tl+1) * I  (fp16, exact)
    w_ext = nc.dram_tensor(
        "w", [C, TH * C], f16, kind="ExternalInput"
    ).ap()
    # Output: one byte-plane per (group, half): bit tl = keep at t=h*4+tl.
    out_ext = nc.dram_tensor(
        "out", [NG * 2, C, F], u8, kind="ExternalOutput"
    ).ap()

    with tile.TileContext(nc) as tc:
        with tc.tile_pool(name="pool", bufs=2) as pool, tc.tile_pool(
            name="psum", bufs=2, space="PSUM"
        ) as ppool:
            wt = pool.tile([C, TH * C], f16, tag="w", bufs=1)
            # load pack weights on the Scalar queue so the first x chunk
            # owns the Sync queue from cycle 0
            nc.scalar.dma_start(out=wt, in_=w_ext)
            # Per-group membrane carry m = tau*u, persists across halves.
            mt = [
                pool.tile([C, F], i16, tag=f"m{g}", bufs=1, name=f"m{g}")
                for g in range(NG)
            ]
            xc = {}
            for h in range(2):
                for pair in range(NG // 2):
                    gs = (2 * pair, 2 * pair + 1)
                    for g in gs:
                        xc[g] = pool.tile(
                            [C, TH * F], i16, tag="x", bufs=6, name=f"x{h}_{g}"
                        )
                    # breadth-first quarter loads
                    for tl in range(TH):
                        for g in gs:
                            lo = tl * F
                            nc.sync.dma_start(
                                out=xc[g][:, lo : lo + F],
                                in_=x_ext[g * 2 + h, :, lo : lo + F],
                            )
                    pk = {
                        g: ppool.tile([C, F], f32, tag="pk", name=f"pk{h}_{g}")
                        for g in gs
                    }
                    for tl in range(TH):
                        t = h * TH + tl
                        vs = {g: xc[g][:, tl * F : (tl + 1) * F] for g in gs}
                        gt = {
                            g: pool.tile(
                                [C, F], f16, tag="g", bufs=8, name=f"g{t}_{g}"
                            )
                            for g in gs
                        }
                        if t > 0:
                            for g in gs:
                                # v = m + q_t (in place; i16 2x)
                                nc.vector.tensor_tensor(
                                    out=vs[g], in0=mt[g], in1=vs[g], op=op.add
                                )
                        for g in gs:
                            # keep-gate with tau folded in: {0,0.5} fp16 (4x)
                            nc.vector.tensor_scalar(
                                out=gt[g], in0=vs[g], scalar1=THI, scalar2=0.5,
                                op0=op.is_le, op1=op.mult,
                            )
                        if t < T - 1:
                            for g in gs:
                                # m = v * g  (reset + tau; i16 x fp16 2x)
                                nc.vector.tensor_tensor(
                                    out=mt[g], in0=vs[g], in1=gt[g], op=op.mult
                                )
                        for g in gs:
                            # pack += 2^(tl+1) * g  (PE, f32 PSUM, exact;
                            # moving free dim capped at 512)
                            for j in range(0, F, 512):
                                nc.tensor.matmul(
                                    pk[g][:, j : j + 512],
                                    wt[:, tl * C : (tl + 1) * C],
                                    gt[g][:, j : j + 512],
                                    start=(tl == 0),
                                    stop=(tl == TH - 1),
                                )
                    for g in gs:
                        pu = pool.tile(
                            [C, F], u8, tag="pu", bufs=4, name=f"p{h}{g}"
                        )
                        nc.scalar.copy(out=pu, in_=pk[g])
                        nc.scalar.dma_start(out=out_ext[g * 2 + h], in_=pu)
    nc.compile()
    return nc


def _run(x: np.ndarray, trace: bool = False, tmpdir=None):
    from concourse.bass_utils import run_bass_kernel_spmd

    if "nc" not in _cache:
        _cache["nc"] = _build_nc()
    nc = _cache["nc"]
    x = np.asarray(x)
    q = np.clip(np.rint(x * np.float32(1.0 / SCALE)), -32768, 32767).astype(
        np.int16
    )
    # q[b=(g*GB+bl), t=(h*TH+tl), c, hw] -> [core, g, h, c, tl, bl, hw]
    q6 = q.reshape(N_CORES, NG, GB, 2, TH, C, HW)
    q_shuf = np.ascontiguousarray(q6.transpose(0, 1, 3, 5, 4, 2, 6)).reshape(
        N_CORES, NG * 2, C, TH * GB * HW
    )
    w = np.zeros((C, TH * C), dtype=np.float16)
    for tl in range(TH):
        w[np.arange(C), tl * C + np.arange(C)] = np.float16(2.0 ** (tl + 1))
    in_maps = [{"x": q_shuf[i], "w": w} for i in range(N_CORES)]
    res = run_bass_kernel_spmd(
        nc, in_maps, core_ids=list(range(N_CORES)), trace=trace, tmpdir=tmpdir
    )
    _cache["last_results"] = res
    outs = [res.results[i]["out"] for i in range(N_CORES)]
    # bytes [core, g*2+h, c, (bl hw)]; bit tl = keep at t = h*4+tl
    by = np.stack(outs, axis=0).reshape(N_CORES, NG, 2, 1, C, GB, HW)
    by = by.astype(np.uint8)
    tl_idx = np.arange(TH, dtype=np.uint8).reshape(1, 1, 1, TH, 1, 1, 1)
    keep = (by >> tl_idx) & np.uint8(1)           # [core, g, h, tl, c, bl, hw]
    spk = (1 - keep).astype(np.float32)
    out = spk.transpose(0, 1, 5, 2, 3, 4, 6).reshape(B, T, C, HW)
    return np.ascontiguousarray(out).reshape(B, T, C, 32, 32)


def kernel(x: np.ndarray) -> np.ndarray:
    return _run(x, trace=False)
